# revision 4
# baseline (speedup 1.0000x reference)
"""KPlane density field kernel for 8 Trainium2 NeuronCores.

Strategy (data-parallel over points, tables replicated per core):
  - sigma = exp(a*F0 + b*F1), [a,b] = w1@w2 (no hidden activation -> MLP
    collapses to a 2-vector), F_c = prod over 6 planes of bilinear samples.
  - Bilinear sample via difference tables: for each plane build a packed
    DRAM table T[cell] = [P,Dx,Dy,Dxy]_c0 ++ [P,Dx,Dy,Dxy]_c1 (32B/cell),
    then B_c = P + fx*Dx + fy*Dy + fx*fy*Dxy at cell (iy*512+ix).
  - Gather: one 32B indirect-DMA descriptor per (point, plane).
  - Offsets are PE-transposed per 128-tile to cancel the descriptor
    iteration order (offsets read partition-minor, output written
    partition-major).
"""
import sys

sys.path.insert(0, "/opt/trn_rl_repo")

import numpy as np

N_CORES = 8
N_RAYS = 65536
N_SAMP = 64
N_PTS = N_RAYS * N_SAMP            # 4194304
PTS_PER_CORE = N_PTS // N_CORES    # 524288
RES = [512, 512, 512, 300]
PAIRS = [(0, 1), (0, 2), (0, 3), (1, 2), (1, 3), (2, 3)]
W = 512                            # width of every plane
HS = [512, 512, 300, 512, 300, 300]  # height (rows) of each plane
D = 8                              # floats per packed table cell

T = 128                            # points per partition per tile (transpose size)
G = 4                              # tiles per group
N_GRP = PTS_PER_CORE // (128 * T * G)  # 8 groups
NG = T * G                         # stream length per partition per group (512)

_nc_cache = {}


def _build_nc():
    if "nc" in _nc_cache:
        return _nc_cache["nc"]
    import concourse.bass as bass
    import concourse.tile as tile
    from concourse import bacc, mybir
    from concourse.bass import IndirectOffsetOnAxis

    f32 = mybir.dt.float32
    u32 = mybir.dt.uint32
    Alu = mybir.AluOpType
    Act = mybir.ActivationFunctionType

    nc = bacc.Bacc("TRN2", target_bir_lowering=False, debug=False,
                   enable_asserts=False, num_devices=N_CORES)

    pts_d = nc.dram_tensor("pts", [PTS_PER_CORE * 4], f32, kind="ExternalInput").ap()
    plane_d = [nc.dram_tensor(f"plane{i}", [2, HS[i], W], f32, kind="ExternalInput").ap()
               for i in range(6)]
    w1_d = nc.dram_tensor("w1", [2 * 64], f32, kind="ExternalInput").ap()
    w2_d = nc.dram_tensor("w2", [64], f32, kind="ExternalInput").ap()
    ident_d = nc.dram_tensor("ident", [128, 128], f32, kind="ExternalInput").ap()
    sig_d = nc.dram_tensor("sigma", [N_GRP, 128, NG], f32, kind="ExternalOutput").ap()

    table_d = [nc.dram_tensor(f"table{i}", [HS[i] * W * D], f32, kind="Internal").ap()
               for i in range(6)]

    with tile.TileContext(nc) as tc:
        # ---------------- phase A: packed difference tables -------------
        with tc.tile_pool(name="tbl", bufs=3) as tp, \
             tc.tile_pool(name="tbl_slab", bufs=3) as sp:
            for pl in range(6):
                H = HS[pl]
                r0 = 0
                while r0 < H:
                    rows = min(128, H - r0)
                    srows = min(128, H - 1 - r0)  # shifted rows available
                    slab = sp.tile([128, W * D], f32, tag="slab")
                    for c in range(2):
                        al = tp.tile([128, W], f32, tag="al")
                        nc.sync.dma_start(
                            al[:rows, :], plane_d[pl][c, r0:r0 + rows, :])
                        sh = tp.tile([128, W], f32, tag="sh")
                        if srows > 0:
                            nc.sync.dma_start(
                                sh[:srows, :], plane_d[pl][c, r0 + 1:r0 + 1 + srows, :])
                        o = 4 * c
                        slab_v = slab[:].rearrange("p (t d) -> p t d", d=D)
                        # P
                        nc.vector.tensor_copy(slab_v[:, :, o + 0], al[:, :])
                        # Dx = P[:, x+1] - P[:, x]  (last col garbage-unused)
                        nc.vector.tensor_tensor(
                            slab_v[:, 0:W - 1, o + 1], al[:, 1:W], al[:, 0:W - 1],
                            Alu.subtract)
                        # Dy = Pshift - P
                        nc.vector.tensor_tensor(
                            slab_v[:, :, o + 2], sh[:, :], al[:, :], Alu.subtract)
                        # Dxy = Dy[:, x+1] - Dy[:, x]
                        nc.vector.tensor_tensor(
                            slab_v[:, 0:W - 1, o + 3],
                            slab_v[:, 1:W, o + 2], slab_v[:, 0:W - 1, o + 2],
                            Alu.subtract)
                    nc.sync.dma_start(
                        table_d[pl].rearrange("(r x) -> r x", x=W * D)[r0:r0 + rows, :],
                        slab[:rows, :])
                    r0 += 128

        # ---------------- phase B: a,b = w1 @ w2, broadcast -------------
        with tc.tile_pool(name="ab", bufs=1) as abp, \
             tc.tile_pool(name="abps", bufs=1, space="PSUM") as abpp:
            w1_sb = abp.tile([1, 128], f32)
            nc.sync.dma_start(w1_sb[:], w1_d.rearrange("(a b) -> a b", a=1))
            w2_sb = abp.tile([1, 64], f32)
            nc.sync.dma_start(w2_sb[:], w2_d.rearrange("(a b) -> a b", a=1))
            m = abp.tile([1, 128], f32)
            # w1 layout is [2,64] row-major: w1[0,:]=first 64, w1[1,:]=last 64
            nc.vector.tensor_tensor(m[:, 0:64], w1_sb[:, 0:64], w2_sb[:], Alu.mult)
            nc.vector.tensor_tensor(m[:, 64:128], w1_sb[:, 64:128], w2_sb[:], Alu.mult)
            ab = abp.tile([1, 2], f32)
            nc.vector.tensor_reduce(ab[:, 0:1], m[:, 0:64],
                                    mybir.AxisListType.X, Alu.add)
            nc.vector.tensor_reduce(ab[:, 1:2], m[:, 64:128],
                                    mybir.AxisListType.X, Alu.add)
            ones1 = abp.tile([1, 128], f32)
            nc.vector.memset(ones1[:], 1.0)
            ab_ps = abpp.tile([128, 2], f32)
            nc.tensor.matmul(ab_ps[:], ones1[:], ab[:])
            ab_sb = abp.tile([128, 2], f32)
            nc.vector.tensor_copy(ab_sb[:], ab_ps[:])

            ident_sb = abp.tile([128, 128], f32)
            nc.sync.dma_start(ident_sb[:], ident_d)

            # ------------- phase C: main point loop -------------------
            pts_v = pts_d.rearrange("(g k p t c) -> g k p (t c)",
                                    g=N_GRP, k=G, p=128, t=T)
            with tc.tile_pool(name="io", bufs=2) as iop, \
                 tc.tile_pool(name="coord", bufs=2) as cop, \
                 tc.tile_pool(name="pp", bufs=2, space="PSUM") as ppp, \
                 tc.tile_pool(name="gath", bufs=2) as gp, \
                 tc.tile_pool(name="lerp", bufs=2) as lp, \
                 tc.tile_pool(name="lerp1", bufs=1) as lp1, \
                 tc.tile_pool(name="bacc", bufs=1) as bp:
                for g in range(N_GRP):
                    pts_sb = iop.tile([128, G * T * 4], f32, tag="pts")
                    for k in range(G):
                        nc.sync.dma_start(pts_sb[:, k * T * 4:(k + 1) * T * 4],
                                          pts_v[g, k])
                    pts4 = pts_sb[:].rearrange("p (n c) -> p n c", c=4)
                    # fractions + integer parts per coordinate
                    frac = cop.tile([128, 4 * NG], f32, tag="frac")
                    ipart = cop.tile([128, 4 * NG], f32, tag="ipart")
                    fr4 = frac[:].rearrange("p (c n) -> p c n", c=4)
                    ip4 = ipart[:].rearrange("p (c n) -> p c n", c=4)
                    x_t = cop.tile([128, NG], f32, tag="xt")
                    r_t = cop.tile([128, NG], f32, tag="rt")
                    c_t = cop.tile([128, NG], f32, tag="ct")
                    for d in range(4):
                        sc = float(RES[d] - 1)
                        nc.vector.tensor_scalar(x_t[:], pts4[:, :, d], sc, None,
                                                Alu.mult)
                        # r = round-to-nearest(x) via 2^23 magic
                        nc.vector.tensor_scalar(r_t[:], x_t[:], 8388608.0,
                                                -8388608.0, Alu.add, Alu.add)
                        # floor fixup: i = r - (x < r)
                        nc.vector.tensor_tensor(c_t[:], x_t[:], r_t[:], Alu.is_lt)
                        nc.vector.tensor_tensor(ip4[:, d, :], r_t[:], c_t[:],
                                                Alu.subtract)
                        nc.vector.tensor_tensor(fr4[:, d, :], x_t[:], ip4[:, d, :],
                                                Alu.subtract)

                    bt = bp.tile([128, 6 * NG * 2], f32, tag="bt")
                    bt6 = bt[:].rearrange("p (i n c) -> p i n c", i=6, c=2)
                    for pl, (j, kk) in enumerate(PAIRS):
                        # flat cell index = iy*512 + ix  (f32, exact)
                        flat = cop.tile([128, NG], f32, tag="flat")
                        nc.vector.scalar_tensor_tensor(
                            flat[:], ip4[:, kk, :], 512.0, ip4[:, j, :],
                            Alu.mult, Alu.add)
                        # transpose each 128-tile via PE, cast to uint32
                        ps = ppp.tile([128, NG], f32, tag="ps")
                        for k in range(G):
                            nc.tensor.transpose(ps[:, k * T:(k + 1) * T],
                                                flat[:, k * T:(k + 1) * T],
                                                ident_sb[:])
                        offs = cop.tile([128, NG], u32, tag="offs")
                        nc.vector.tensor_copy(offs[:], ps[:])
                        # gather 32B cells
                        v = gp.tile([128, NG * D], f32, tag="v")
                        for k in range(G):
                            nc.gpsimd.indirect_dma_start(
                                v[:, k * T * D:(k + 1) * T * D].rearrange(
                                    "p (t d) -> p t d", d=D),
                                None,
                                table_d[pl].rearrange("(c d) -> c d", d=D),
                                IndirectOffsetOnAxis(
                                    ap=offs[:, k * T:(k + 1) * T], axis=0),
                            )
                        # weights [1, fx, fy, fx*fy] per channel
                        w8 = lp.tile([128, NG * D], f32, tag="w8")
                        w8v = w8[:].rearrange("p (n d) -> p n d", d=D)
                        if g == 0 and pl == 0:
                            nc.vector.memset(w8[:], 1.0)
                        fj = fr4[:, j, :]
                        fk = fr4[:, kk, :]
                        for o in (1, 5):
                            nc.vector.tensor_copy(w8v[:, :, o], fj)
                        for o in (2, 6):
                            nc.vector.tensor_copy(w8v[:, :, o], fk)
                        nc.vector.tensor_tensor(w8v[:, :, 3], fj, fk, Alu.mult)
                        nc.vector.tensor_copy(w8v[:, :, 7], w8v[:, :, 3])
                        # U = V * W8 (in place); tree-sum into B[., n, c]
                        nc.vector.tensor_tensor(v[:], v[:], w8[:], Alu.mult)
                        uv = v[:].rearrange("p (n d) -> p n d", d=D)
                        s1 = lp1.tile([128, NG * 4], f32, tag="s1")
                        s1v = s1[:].rearrange("p (n d) -> p n d", d=4)
                        nc.vector.tensor_tensor(s1v[:], uv[:, :, 0::2],
                                                uv[:, :, 1::2], Alu.add)
                        nc.vector.tensor_tensor(bt6[:, pl, :, :], s1v[:, :, 0::2],
                                                s1v[:, :, 1::2], Alu.add)
                    # product over 6 planes
                    f01 = bp.tile([128, NG * 2], f32, tag="f01")
                    fv = f01[:].rearrange("p (n c) -> p n c", c=2)
                    nc.vector.tensor_tensor(fv[:], bt6[:, 0], bt6[:, 1], Alu.mult)
                    for pl in range(2, 6):
                        nc.vector.tensor_tensor(fv[:], fv[:], bt6[:, pl], Alu.mult)
                    # sigma = exp(a*F0 + b*F1)
                    sarg = bp.tile([128, NG], f32, tag="sarg")
                    nc.vector.tensor_scalar(sarg[:], fv[:, :, 1], ab_sb[:, 1:2],
                                            None, Alu.mult)
                    nc.vector.scalar_tensor_tensor(
                        sarg[:], fv[:, :, 0], ab_sb[:, 0:1], sarg[:],
                        Alu.mult, Alu.add)
                    sig_sb = iop.tile([128, NG], f32, tag="sig")
                    nc.scalar.activation(sig_sb[:], sarg[:], Act.Exp)
                    nc.sync.dma_start(sig_d[g], sig_sb[:])

    nc.compile()
    _nc_cache["nc"] = nc
    return nc


def kernel(pts, plane0, plane1, plane2, plane3, plane4, plane5, w1, w2, aabb):
    from concourse import bass_utils
    try:
        import axon_shim  # noqa: F401
    except ImportError:
        _install_shim()

    nc = _build_nc()

    pts = np.asarray(pts, dtype=np.float32)
    planes = [np.ascontiguousarray(np.asarray(p, dtype=np.float32))
              for p in (plane0, plane1, plane2, plane3, plane4, plane5)]
    aabb = np.asarray(aabb, dtype=np.float32)

    # fold aabb normalization into pts on host iff non-trivial (it is [0,1]
    # for this problem; reference: x = (pts-lo)/(hi-lo) * (R-1))
    lo, hi = aabb[0], aabb[1]
    p = pts.reshape(-1, 4)
    if not (np.allclose(lo, 0.0) and np.allclose(hi, 1.0)):
        p = (p - lo) / (hi - lo)
    p = np.ascontiguousarray(p, dtype=np.float32)

    ident = np.eye(128, dtype=np.float32)
    w1f = np.ascontiguousarray(np.asarray(w1, np.float32).reshape(-1))
    w2f = np.ascontiguousarray(np.asarray(w2, np.float32).reshape(-1))

    in_maps = []
    for c in range(N_CORES):
        sl = p[c * PTS_PER_CORE:(c + 1) * PTS_PER_CORE].reshape(-1)
        m = {"pts": np.ascontiguousarray(sl), "w1": w1f, "w2": w2f,
             "ident": ident}
        for i in range(6):
            m[f"plane{i}"] = planes[i]
        in_maps.append(m)

    res = bass_utils.run_bass_kernel_spmd(
        nc, in_maps, core_ids=list(range(N_CORES)),
        trace=bool(int(__import__("os").environ.get("KPLANE_TRACE", "0"))))
    if res.exec_time_ns is not None:
        kernel.last_exec_time_ns = res.exec_time_ns

    outs = []
    for c in range(N_CORES):
        s = res.results[c]["sigma"]          # [N_GRP, 128, NG]
        s = s.reshape(N_GRP, 128, G, T)      # [g, p, k, t]
        s = s.transpose(0, 2, 1, 3)          # [g, k, p, t]
        outs.append(s.reshape(-1))
    out = np.concatenate(outs)
    return out.reshape(N_RAYS, N_SAMP, 1).astype(np.float32)


kernel.last_exec_time_ns = None


def _install_shim():
    """Self-contained antenv.axon_hooks shim (for fresh grading dirs)."""
    import contextlib
    import ctypes
    import types

    if "antenv.axon_hooks" in sys.modules:
        return
    hook = None
    try:
        lib = ctypes.CDLL("/opt/axon/libaxon_pjrt.so")
        if hasattr(lib, "axon_start_nrt_profile"):
            lib.axon_start_nrt_profile.argtypes = [
                ctypes.POINTER(ctypes.c_int64), ctypes.c_size_t]
            lib.axon_start_nrt_profile.restype = ctypes.c_int64
            lib.axon_stop_nrt_profile.argtypes = [ctypes.c_char_p]
            lib.axon_stop_nrt_profile.restype = ctypes.c_int64

            @contextlib.contextmanager
            def hook(output_dir, device_ids):
                import jax
                jax.devices()
                if device_ids:
                    ids = (ctypes.c_int64 * len(device_ids))(*device_ids)
                    rc = lib.axon_start_nrt_profile(ids, len(device_ids))
                else:
                    rc = lib.axon_start_nrt_profile(None, 0)
                if rc != 0:
                    raise RuntimeError(f"axon_start_nrt_profile rc={rc}")
                try:
                    yield
                finally:
                    lib.axon_stop_nrt_profile(str(output_dir).encode())
    except OSError:
        pass
    mod = types.ModuleType("antenv.axon_hooks")
    mod.get_axon_ntff_profile_hook = lambda: hook
    mod.set_axon_ntff_profile_hook = lambda h: None
    sys.modules["antenv.axon_hooks"] = mod
    try:
        import antenv
        antenv.axon_hooks = mod
    except ImportError:
        pass


if __name__ == "__main__":
    rng = np.random.default_rng(0)
    inputs = {
        "pts": rng.random((N_RAYS, N_SAMP, 4), np.float32),
        "w1": rng.standard_normal((2, 64)).astype(np.float32) * 0.1,
        "w2": rng.standard_normal((64, 1)).astype(np.float32) * 0.1,
        "aabb": np.array([[0, 0, 0, 0], [1, 1, 1, 1]], np.float32),
    }
    for i, (j, k) in enumerate(PAIRS):
        inputs[f"plane{i}"] = (rng.random((2, RES[k], RES[j]), np.float32)
                               * 0.05 + 0.1)
    out = kernel(**inputs)
    print(out.shape, out.dtype, kernel.last_exec_time_ns)
